# revision 5
# baseline (speedup 1.0000x reference)
"""Behler-Parrinello NN (moe_routing) Trainium2 kernel.

Strategy:
  - Data-parallel over batch B=512 across 8 NeuronCores (64 rows each).
  - Atoms are routed: sorted by type into type-pure "chunks" of 8 atoms
    (x 64 batch rows = 512 tokens), padded with zero-atoms to chunk
    boundaries.  Each chunk runs through its own type's MLP weights, so
    compute is 1/T of the reference's all-types evaluation.
  - Per 128x512 SBUF tile we stack two chunks (top/bottom 64 partitions)
    and use tensor-engine tile_position packing so two 64x64 matmuls run
    concurrently in the 128x128 array.
  - Layer 3 (H2 -> 1) matmuls accumulate into a single persistent PSUM
    bank across the whole kernel, which implements the sum over atoms for
    free; the per-batch reduction finishes on the host (tiny).
  - silu(W x + b) is fused on ScalarE via activation(Silu, bias=...).
"""

import os
import sys

for _p in ("/opt/trn_rl_repo", "/root/.axon_site/_ro/trn_rl_repo"):
    if os.path.isdir(_p) and _p not in sys.path:
        sys.path.insert(0, _p)

import numpy as np

import concourse.bass as bass
import concourse.tile as tile
from concourse import bacc, mybir
from concourse.bass import ts
from concourse.bass_utils import run_bass_kernel_spmd

B, N, F, T, H1, H2 = 512, 2048, 64, 4, 64, 32
NCORES = 8
BC = B // NCORES          # 64 batch rows per core
CA = 8                    # atoms per chunk; CA * BC = 512 tokens per chunk
F32 = mybir.dt.float32

# test.py can read these after a traced run
LAST_EXEC_NS = None
LAST_RESULTS = None


def _ensure_ntff_hook():
    """This image's antenv lacks axon_hooks; synthesize it and install the
    ctypes NTFF profile hook from trn_agent_boot so trace=True works."""
    import importlib.util
    import types

    if importlib.util.find_spec("antenv.axon_hooks") is not None:
        return
    import antenv

    mod = types.ModuleType("antenv.axon_hooks")
    mod._hook = None
    mod.set_axon_ntff_profile_hook = lambda h: setattr(mod, "_hook", h)
    mod.get_axon_ntff_profile_hook = lambda: mod._hook
    sys.modules["antenv.axon_hooks"] = mod
    antenv.axon_hooks = mod
    try:
        from trn_agent_boot.trn_boot import _ntff_profile_via_ctypes

        mod._hook = _ntff_profile_via_ctypes("/opt/axon/libaxon_pjrt.so")
    except Exception as e:  # degrade to no-trace
        print(f"ntff hook install failed: {e}", file=sys.stderr)


def _chunk_schedule(an):
    """Sort atoms by type, pad each type to a chunk multiple, pad chunk count
    to a multiple of 4 (one quad = 4 chunks).  Returns (slots, ctype,
    counts, pad_counts): slots is [nchunks*CA] atom indices with -1 = pad."""
    order = np.argsort(an, kind="stable")
    counts = np.bincount(an, minlength=T).astype(np.int64)
    slots = []
    ctype = []
    pad_counts = np.zeros(T, dtype=np.int64)
    pos = 0
    for t in range(T):
        idx = order[pos : pos + counts[t]]
        pos += counts[t]
        nch = (counts[t] + CA - 1) // CA
        padded = np.full(nch * CA, -1, dtype=np.int64)
        padded[: counts[t]] = idx
        pad_counts[t] += nch * CA - counts[t]
        slots.append(padded)
        ctype.extend([t] * int(nch))
    while len(ctype) % 4 != 0:
        slots.append(np.full(CA, -1, dtype=np.int64))
        pad_counts[T - 1] += CA
        ctype.append(T - 1)
    return np.concatenate(slots), np.array(ctype, dtype=np.int64), counts, pad_counts


def gen_bass(nchunks, ctype):
    """Build the per-core Bass kernel.  ctype (len nchunks, multiple of 4)
    is baked in at compile time."""
    npairs = nchunks // 2
    nquads = nchunks // 4
    Silu = mybir.ActivationFunctionType.Silu

    nc = bacc.Bacc(None, target_bir_lowering=False)
    xt3 = nc.dram_tensor("xt3", [npairs, 128, CA * BC], F32, kind="ExternalInput")
    w0d = nc.dram_tensor("w0s", [128, T * H1], F32, kind="ExternalInput")
    w1d = nc.dram_tensor("w1s", [128, T * H2], F32, kind="ExternalInput")
    w2d = nc.dram_tensor("w2s", [128, T * 32], F32, kind="ExternalInput")
    b0d = nc.dram_tensor("b0p", [128, npairs], F32, kind="ExternalInput")
    b1d = nc.dram_tensor("b1q", [128, nquads], F32, kind="ExternalInput")
    outd = nc.dram_tensor("out", [4, CA * BC], F32, kind="ExternalOutput")

    with tile.TileContext(nc) as tc:
        with (
            tc.tile_pool(name="consts", bufs=1) as cpool,
            tc.tile_pool(name="xp", bufs=6) as xpool,
            tc.tile_pool(name="h1p", bufs=3) as h1pool,
            tc.tile_pool(name="h2p", bufs=3) as h2pool,
            tc.tile_pool(name="outp", bufs=1) as opool,
            tc.tile_pool(name="ps1", bufs=2, space="PSUM") as ps1pool,
            tc.tile_pool(name="ps2", bufs=2, space="PSUM") as ps2pool,
            tc.tile_pool(name="ps3", bufs=2, space="PSUM") as ps3pool,
        ):
            w0t = cpool.tile([128, T * H1], F32)
            nc.sync.dma_start(w0t[:], w0d[:])
            w1t = cpool.tile([128, T * H2], F32)
            nc.sync.dma_start(w1t[:], w1d[:])
            w2t = cpool.tile([128, T * 32], F32)
            nc.sync.dma_start(w2t[:], w2d[:])
            b0t = cpool.tile([128, npairs], F32)
            nc.sync.dma_start(b0t[:], b0d[:])
            b1t = cpool.tile([128, nquads], F32)
            nc.sync.dma_start(b1t[:], b1d[:])

            # SBUF accumulator for the atom-sum; row blocks of 32, only the
            # first row of each block is nonzero (w2 lanes are zero-padded)
            acc = opool.tile([128, 512], F32)
            nc.vector.memset(acc[:], 0.0)

            for q in range(nquads):
                tA, tB, tC, tD = (int(t) for t in ctype[4 * q : 4 * q + 4])
                x0 = xpool.tile([128, 512], F32, tag="x")
                nc.sync.dma_start(x0[:], xt3[2 * q])
                x1 = xpool.tile([128, 512], F32, tag="x")
                nc.sync.dma_start(x1[:], xt3[2 * q + 1])

                # L1: h1_pre = w0[t].T.T @ x   (2 concurrent 64x64 lanes/tile)
                ps1 = ps1pool.tile([128, 1024], F32)
                nc.tensor.matmul(ps1[0:64, 0:512], w0t[0:64, ts(tA, H1)],
                                 x0[0:64, :], start=True, stop=True,
                                 tile_position=(0, 0))
                nc.tensor.matmul(ps1[64:128, 0:512], w0t[64:128, ts(tB, H1)],
                                 x0[64:128, :], start=True, stop=True,
                                 tile_position=(64, 64))
                nc.tensor.matmul(ps1[0:64, 512:1024], w0t[0:64, ts(tC, H1)],
                                 x1[0:64, :], start=True, stop=True,
                                 tile_position=(0, 0))
                nc.tensor.matmul(ps1[64:128, 512:1024], w0t[64:128, ts(tD, H1)],
                                 x1[64:128, :], start=True, stop=True,
                                 tile_position=(64, 64))

                h1 = h1pool.tile([128, 1024], F32)
                if (tA, tB) == (tC, tD):
                    nc.scalar.activation(h1[:], ps1[:], Silu,
                                         bias=b0t[:, 2 * q : 2 * q + 1])
                else:
                    nc.scalar.activation(h1[:, 0:512], ps1[:, 0:512], Silu,
                                         bias=b0t[:, 2 * q : 2 * q + 1])
                    nc.scalar.activation(h1[:, 512:1024], ps1[:, 512:1024], Silu,
                                         bias=b0t[:, 2 * q + 1 : 2 * q + 2])

                # L2: four 64x32 lanes -> one fully packed psum bank
                ps2 = ps2pool.tile([128, 512], F32)
                nc.tensor.matmul(ps2[64:96, :], w1t[0:64, ts(tA, H2)],
                                 h1[0:64, 0:512], start=True, stop=True,
                                 tile_position=(0, 64))
                nc.tensor.matmul(ps2[0:32, :], w1t[64:128, ts(tB, H2)],
                                 h1[64:128, 0:512], start=True, stop=True,
                                 tile_position=(64, 0))
                nc.tensor.matmul(ps2[96:128, :], w1t[0:64, ts(tC, H2)],
                                 h1[0:64, 512:1024], start=True, stop=True,
                                 tile_position=(0, 96))
                nc.tensor.matmul(ps2[32:64, :], w1t[64:128, ts(tD, H2)],
                                 h1[64:128, 512:1024], start=True, stop=True,
                                 tile_position=(64, 32))

                h2 = h2pool.tile([128, 512], F32)
                nc.scalar.activation(h2[:], ps2[:], Silu, bias=b1t[:, q : q + 1])

                # L3: four 32x32 lanes (w2 zero-padded to 32 cols) -> fresh
                # psum bank per quad, accumulated into SBUF on VectorE
                ps3 = ps3pool.tile([128, 512], F32, tag="ps3")
                nc.tensor.matmul(ps3[64:96, :], w2t[0:32, ts(tB, 32)],
                                 h2[0:32, :], start=True, stop=True,
                                 tile_position=(0, 64))
                nc.tensor.matmul(ps3[96:128, :], w2t[32:64, ts(tD, 32)],
                                 h2[32:64, :], start=True, stop=True,
                                 tile_position=(32, 96))
                nc.tensor.matmul(ps3[0:32, :], w2t[64:96, ts(tA, 32)],
                                 h2[64:96, :], start=True, stop=True,
                                 tile_position=(64, 0))
                nc.tensor.matmul(ps3[32:64, :], w2t[96:128, ts(tC, 32)],
                                 h2[96:128, :], start=True, stop=True,
                                 tile_position=(96, 32))
                nc.vector.tensor_add(out=acc[:], in0=acc[:], in1=ps3[:])

            for i, p in enumerate((0, 32, 64, 96)):
                nc.sync.dma_start(outd[i : i + 1, :], acc[p : p + 1, :])
    nc.finalize()
    return nc


def _prep_core_x(x_c, slots, mask, npairs):
    """[BC, N, F] full-precision batch shard -> [npairs, 128, CA*BC] tiles.
    Tile p partition h*F+f, column a*BC+b = x_c[b, slots[(2p+h)*CA+a], f]."""
    xg = x_c[:, np.where(mask, slots, 0), :]          # [BC, NS, F]
    xg[:, ~mask, :] = 0.0
    nchunks = slots.shape[0] // CA
    xg = np.ascontiguousarray(xg.transpose(1, 2, 0))  # [NS, F, BC]
    xg = xg.reshape(nchunks, CA, F, BC).transpose(0, 2, 1, 3)  # [ch, F, CA, BC]
    return np.ascontiguousarray(xg).reshape(npairs, 2 * F, CA * BC)


def kernel(x, atomic_numbers, w0, b0, w1, b1, w2, b2, trace=False):
    global LAST_EXEC_NS, LAST_RESULTS
    x = np.asarray(x, dtype=np.float32)
    an = np.asarray(atomic_numbers).astype(np.int64)
    w0 = np.asarray(w0, dtype=np.float32)
    b0 = np.asarray(b0, dtype=np.float32)
    w1 = np.asarray(w1, dtype=np.float32)
    b1 = np.asarray(b1, dtype=np.float32)
    w2 = np.asarray(w2, dtype=np.float32)
    b2 = np.asarray(b2, dtype=np.float32)

    slots, ctype, counts, pad_counts = _chunk_schedule(an)
    nchunks = len(ctype)
    npairs, nquads = nchunks // 2, nchunks // 4
    mask = slots >= 0

    # --- device-side weight/bias layouts (shared across cores) ---
    w0s = np.zeros((128, T * H1), dtype=np.float32)
    w1s = np.zeros((128, T * H2), dtype=np.float32)
    w2s = np.zeros((128, T * 32), dtype=np.float32)
    for t in range(T):
        w0s[0:64, t * H1 : (t + 1) * H1] = w0[t].T
        w0s[64:128, t * H1 : (t + 1) * H1] = w0[t].T
        w1s[0:64, t * H2 : (t + 1) * H2] = w1[t].T
        w1s[64:128, t * H2 : (t + 1) * H2] = w1[t].T
        for g in range(4):
            w2s[32 * g : 32 * g + 32, t * 32] = w2[t, 0, :]
    b0p = np.zeros((128, npairs), dtype=np.float32)
    for p in range(npairs):
        b0p[0:64, p] = b0[ctype[2 * p]]
        b0p[64:128, p] = b0[ctype[2 * p + 1]]
    b1q = np.zeros((128, nquads), dtype=np.float32)
    for q in range(nquads):
        tA, tB, tC, tD = ctype[4 * q : 4 * q + 4]
        b1q[0:32, q] = b1[tB]
        b1q[32:64, q] = b1[tD]
        b1q[64:96, q] = b1[tA]
        b1q[96:128, q] = b1[tC]

    shared = {"w0s": w0s, "w1s": w1s, "w2s": w2s, "b0p": b0p, "b1q": b1q}
    in_maps = []
    for c in range(NCORES):
        xt3 = _prep_core_x(x[c * BC : (c + 1) * BC], slots, mask, npairs)
        in_maps.append({"xt3": xt3, **shared})

    if trace:
        _ensure_ntff_hook()
    nc = gen_bass(nchunks, ctype)
    res = run_bass_kernel_spmd(nc, in_maps, core_ids=list(range(NCORES)),
                               trace=trace)
    LAST_EXEC_NS = res.exec_time_ns
    LAST_RESULTS = res

    # --- host-side unshard + tiny corrections ---
    # device out = sum over streamed tokens of w2 . h2(token); pads
    # contribute e0[t] = w2[t] . silu(w1[t] silu(b0[t]) + b1[t]); real atoms
    # still owe their +b2[t].
    def _silu(v):
        return v / (1.0 + np.exp(-v))

    e0 = np.zeros(T, dtype=np.float64)
    for t in range(T):
        h1v = _silu(b0[t].astype(np.float64))
        h2v = _silu(w1[t].astype(np.float64) @ h1v + b1[t])
        e0[t] = w2[t, 0] @ h2v
    bias_term = float((counts * b2[:, 0].astype(np.float64)).sum())
    pad_term = float((pad_counts * e0).sum())

    out = np.empty(B, dtype=np.float32)
    for c in range(NCORES):
        dev = res.results[c]["out"]                   # [4, CA*BC]
        s = dev.sum(axis=0).reshape(CA, BC).sum(axis=0)
        out[c * BC : (c + 1) * BC] = s + bias_term - pad_term
    return out


# revision 6
# speedup vs baseline: 1.1356x; 1.1356x over previous
"""Behler-Parrinello NN (moe_routing) Trainium2 kernel.

Strategy:
  - Data-parallel over batch B=512 across 8 NeuronCores (64 rows each).
  - Atoms are routed: sorted by type into type-pure "chunks" of 8 atoms
    (x 64 batch rows = 512 tokens), padded with zero-atoms to chunk
    boundaries.  Each chunk runs through its own type's MLP weights, so
    compute is 1/T of the reference's all-types evaluation.
  - Per 128x512 SBUF tile we stack two chunks (top/bottom 64 partitions)
    and use tensor-engine tile_position packing so two 64x64 matmuls run
    concurrently in the 128x128 array.
  - Layer 3 (H2 -> 1) matmuls accumulate into a single persistent PSUM
    bank across the whole kernel, which implements the sum over atoms for
    free; the per-batch reduction finishes on the host (tiny).
  - silu(W x + b) is fused on ScalarE via activation(Silu, bias=...).
"""

import os
import sys

for _p in ("/opt/trn_rl_repo", "/root/.axon_site/_ro/trn_rl_repo"):
    if os.path.isdir(_p) and _p not in sys.path:
        sys.path.insert(0, _p)

import numpy as np

import concourse.bass as bass
import concourse.tile as tile
from concourse import bacc, mybir
from concourse.bass import ts
from concourse.bass_utils import run_bass_kernel_spmd

B, N, F, T, H1, H2 = 512, 2048, 64, 4, 64, 32
NCORES = 8
BC = B // NCORES          # 64 batch rows per core
CA = 8                    # atoms per chunk; CA * BC = 512 tokens per chunk
F32 = mybir.dt.float32

# test.py can read these after a traced run
LAST_EXEC_NS = None
LAST_RESULTS = None


def _ensure_ntff_hook():
    """This image's antenv lacks axon_hooks; synthesize it and install the
    ctypes NTFF profile hook from trn_agent_boot so trace=True works."""
    import importlib.util
    import types

    if importlib.util.find_spec("antenv.axon_hooks") is not None:
        return
    import antenv

    mod = types.ModuleType("antenv.axon_hooks")
    mod._hook = None
    mod.set_axon_ntff_profile_hook = lambda h: setattr(mod, "_hook", h)
    mod.get_axon_ntff_profile_hook = lambda: mod._hook
    sys.modules["antenv.axon_hooks"] = mod
    antenv.axon_hooks = mod
    try:
        from trn_agent_boot.trn_boot import _ntff_profile_via_ctypes

        mod._hook = _ntff_profile_via_ctypes("/opt/axon/libaxon_pjrt.so")
    except Exception as e:  # degrade to no-trace
        print(f"ntff hook install failed: {e}", file=sys.stderr)


def _chunk_schedule(an):
    """Sort atoms by type, pad each type to a chunk multiple, pad chunk count
    to a multiple of 4 (one quad = 4 chunks).  Returns (slots, ctype,
    counts, pad_counts): slots is [nchunks*CA] atom indices with -1 = pad."""
    order = np.argsort(an, kind="stable")
    counts = np.bincount(an, minlength=T).astype(np.int64)
    slots = []
    ctype = []
    pad_counts = np.zeros(T, dtype=np.int64)
    pos = 0
    for t in range(T):
        idx = order[pos : pos + counts[t]]
        pos += counts[t]
        nch = (counts[t] + CA - 1) // CA
        padded = np.full(nch * CA, -1, dtype=np.int64)
        padded[: counts[t]] = idx
        pad_counts[t] += nch * CA - counts[t]
        slots.append(padded)
        ctype.extend([t] * int(nch))
    while len(ctype) % 4 != 0:
        slots.append(np.full(CA, -1, dtype=np.int64))
        pad_counts[T - 1] += CA
        ctype.append(T - 1)
    return np.concatenate(slots), np.array(ctype, dtype=np.int64), counts, pad_counts


def gen_bass(nchunks, ctype):
    """Build the per-core Bass kernel.  ctype (len nchunks, multiple of 4)
    is baked in at compile time."""
    npairs = nchunks // 2
    nquads = nchunks // 4
    Silu = mybir.ActivationFunctionType.Silu

    nc = bacc.Bacc(None, target_bir_lowering=False)
    xt3 = nc.dram_tensor("xt3", [npairs, 128, CA * BC], F32, kind="ExternalInput")
    w0d = nc.dram_tensor("w0s", [128, 16 * 128], F32, kind="ExternalInput")
    w1d = nc.dram_tensor("w1s", [128, 16 * 64], F32, kind="ExternalInput")
    w2d = nc.dram_tensor("w2s", [128, 16 * 64], F32, kind="ExternalInput")
    b0d = nc.dram_tensor("b0p", [128, npairs], F32, kind="ExternalInput")
    b1d = nc.dram_tensor("b1q", [128, nquads], F32, kind="ExternalInput")
    outd = nc.dram_tensor("out", [4, CA * BC], F32, kind="ExternalOutput")

    with tile.TileContext(nc) as tc:
        with (
            tc.tile_pool(name="consts", bufs=1) as cpool,
            tc.tile_pool(name="xp", bufs=6) as xpool,
            tc.tile_pool(name="h1p", bufs=3) as h1pool,
            tc.tile_pool(name="h2p", bufs=3) as h2pool,
            tc.tile_pool(name="outp", bufs=1) as opool,
            tc.tile_pool(name="ps1", bufs=2, space="PSUM") as ps1pool,
            tc.tile_pool(name="ps2", bufs=2, space="PSUM") as ps2pool,
            tc.tile_pool(name="ps3", bufs=2, space="PSUM") as ps3pool,
        ):
            w0t = cpool.tile([128, 16 * 128], F32)
            nc.sync.dma_start(w0t[:], w0d[:])
            w1t = cpool.tile([128, 16 * 64], F32)
            nc.sync.dma_start(w1t[:], w1d[:])
            w2t = cpool.tile([128, 16 * 64], F32)
            nc.sync.dma_start(w2t[:], w2d[:])
            b0t = cpool.tile([128, npairs], F32)
            nc.sync.dma_start(b0t[:], b0d[:])
            b1t = cpool.tile([128, nquads], F32)
            nc.sync.dma_start(b1t[:], b1d[:])

            # SBUF accumulator for the atom-sum; row blocks of 32, only the
            # first row of each block is nonzero (w2 lanes are zero-padded)
            acc = opool.tile([128, 512], F32)
            nc.vector.memset(acc[:], 0.0)

            for q in range(nquads):
                tA, tB, tC, tD = (int(t) for t in ctype[4 * q : 4 * q + 4])
                x0 = xpool.tile([128, 512], F32, tag="x")
                nc.sync.dma_start(x0[:], xt3[2 * q])
                x1 = xpool.tile([128, 512], F32, tag="x")
                nc.sync.dma_start(x1[:], xt3[2 * q + 1])

                # combo ids: block-diag weights [[W_top, 0], [0, W_bot]]
                cAB = tA * 4 + tB
                cCD = tC * 4 + tD

                # L1: one K=128, M=128 matmul per chunk-pair
                ps1 = ps1pool.tile([128, 1024], F32)
                nc.tensor.matmul(ps1[:, 0:512], w0t[:, ts(cAB, 128)],
                                 x0[:, :], start=True, stop=True,
                                 tile_position=(0, 0))
                nc.tensor.matmul(ps1[:, 512:1024], w0t[:, ts(cCD, 128)],
                                 x1[:, :], start=True, stop=True,
                                 tile_position=(0, 0))

                h1 = h1pool.tile([128, 1024], F32)
                if (tA, tB) == (tC, tD):
                    nc.scalar.activation(h1[:], ps1[:], Silu,
                                         bias=b0t[:, 2 * q : 2 * q + 1])
                else:
                    nc.scalar.activation(h1[:, 0:512], ps1[:, 0:512], Silu,
                                         bias=b0t[:, 2 * q : 2 * q + 1])
                    nc.scalar.activation(h1[:, 512:1024], ps1[:, 512:1024], Silu,
                                         bias=b0t[:, 2 * q + 1 : 2 * q + 2])

                # L2: one K=128, M=64 matmul per pair; quarters c0,c1,c2,c3
                ps2 = ps2pool.tile([128, 512], F32)
                nc.tensor.matmul(ps2[0:64, :], w1t[:, ts(cAB, 64)],
                                 h1[:, 0:512], start=True, stop=True,
                                 tile_position=(0, 0))
                nc.tensor.matmul(ps2[64:128, :], w1t[:, ts(cCD, 64)],
                                 h1[:, 512:1024], start=True, stop=True,
                                 tile_position=(0, 64))

                h2 = h2pool.tile([128, 512], F32)
                nc.scalar.activation(h2[:], ps2[:], Silu, bias=b1t[:, q : q + 1])

                # L3: one K=64, M=64 matmul per pair (w2 zero-padded blocks);
                # nonzero output rows: 0, 32, 64, 96
                ps3 = ps3pool.tile([128, 512], F32, tag="ps3")
                nc.tensor.matmul(ps3[0:64, :], w2t[0:64, ts(cAB, 64)],
                                 h2[0:64, :], start=True, stop=True,
                                 tile_position=(0, 0))
                nc.tensor.matmul(ps3[64:128, :], w2t[64:128, ts(cCD, 64)],
                                 h2[64:128, :], start=True, stop=True,
                                 tile_position=(64, 64))
                nc.vector.tensor_add(out=acc[:], in0=acc[:], in1=ps3[:])

            for i, p in enumerate((0, 32, 64, 96)):
                nc.sync.dma_start(outd[i : i + 1, :], acc[p : p + 1, :])
    nc.finalize()
    return nc


def _prep_core_x(x_c, slots, mask, npairs):
    """[BC, N, F] full-precision batch shard -> [npairs, 128, CA*BC] tiles.
    Tile p partition h*F+f, column a*BC+b = x_c[b, slots[(2p+h)*CA+a], f]."""
    xg = x_c[:, np.where(mask, slots, 0), :]          # [BC, NS, F]
    xg[:, ~mask, :] = 0.0
    nchunks = slots.shape[0] // CA
    xg = np.ascontiguousarray(xg.transpose(1, 2, 0))  # [NS, F, BC]
    xg = xg.reshape(nchunks, CA, F, BC).transpose(0, 2, 1, 3)  # [ch, F, CA, BC]
    return np.ascontiguousarray(xg).reshape(npairs, 2 * F, CA * BC)


def kernel(x, atomic_numbers, w0, b0, w1, b1, w2, b2, trace=False):
    global LAST_EXEC_NS, LAST_RESULTS
    x = np.asarray(x, dtype=np.float32)
    an = np.asarray(atomic_numbers).astype(np.int64)
    w0 = np.asarray(w0, dtype=np.float32)
    b0 = np.asarray(b0, dtype=np.float32)
    w1 = np.asarray(w1, dtype=np.float32)
    b1 = np.asarray(b1, dtype=np.float32)
    w2 = np.asarray(w2, dtype=np.float32)
    b2 = np.asarray(b2, dtype=np.float32)

    slots, ctype, counts, pad_counts = _chunk_schedule(an)
    nchunks = len(ctype)
    npairs, nquads = nchunks // 2, nchunks // 4
    mask = slots >= 0

    # --- device-side weight/bias layouts (shared across cores) ---
    w0s = np.zeros((128, 16 * 128), dtype=np.float32)
    w1s = np.zeros((128, 16 * 64), dtype=np.float32)
    w2s = np.zeros((128, 16 * 64), dtype=np.float32)
    for tt in range(T):
        for tb in range(T):
            c = tt * 4 + tb
            w0s[0:64, c * 128 : c * 128 + 64] = w0[tt].T
            w0s[64:128, c * 128 + 64 : c * 128 + 128] = w0[tb].T
            w1s[0:64, c * 64 : c * 64 + 32] = w1[tt].T
            w1s[64:128, c * 64 + 32 : c * 64 + 64] = w1[tb].T
            for half in (0, 64):
                w2s[half : half + 32, c * 64] = w2[tt, 0, :]
                w2s[half + 32 : half + 64, c * 64 + 32] = w2[tb, 0, :]
    b0p = np.zeros((128, npairs), dtype=np.float32)
    for p in range(npairs):
        b0p[0:64, p] = b0[ctype[2 * p]]
        b0p[64:128, p] = b0[ctype[2 * p + 1]]
    b1q = np.zeros((128, nquads), dtype=np.float32)
    for q in range(nquads):
        tA, tB, tC, tD = ctype[4 * q : 4 * q + 4]
        b1q[0:32, q] = b1[tA]
        b1q[32:64, q] = b1[tB]
        b1q[64:96, q] = b1[tC]
        b1q[96:128, q] = b1[tD]

    shared = {"w0s": w0s, "w1s": w1s, "w2s": w2s, "b0p": b0p, "b1q": b1q}
    in_maps = []
    for c in range(NCORES):
        xt3 = _prep_core_x(x[c * BC : (c + 1) * BC], slots, mask, npairs)
        in_maps.append({"xt3": xt3, **shared})

    if trace:
        _ensure_ntff_hook()
    nc = gen_bass(nchunks, ctype)
    res = run_bass_kernel_spmd(nc, in_maps, core_ids=list(range(NCORES)),
                               trace=trace)
    LAST_EXEC_NS = res.exec_time_ns
    LAST_RESULTS = res

    # --- host-side unshard + tiny corrections ---
    # device out = sum over streamed tokens of w2 . h2(token); pads
    # contribute e0[t] = w2[t] . silu(w1[t] silu(b0[t]) + b1[t]); real atoms
    # still owe their +b2[t].
    def _silu(v):
        return v / (1.0 + np.exp(-v))

    e0 = np.zeros(T, dtype=np.float64)
    for t in range(T):
        h1v = _silu(b0[t].astype(np.float64))
        h2v = _silu(w1[t].astype(np.float64) @ h1v + b1[t])
        e0[t] = w2[t, 0] @ h2v
    bias_term = float((counts * b2[:, 0].astype(np.float64)).sum())
    pad_term = float((pad_counts * e0).sum())

    out = np.empty(B, dtype=np.float32)
    for c in range(NCORES):
        dev = res.results[c]["out"]                   # [4, CA*BC]
        s = dev.sum(axis=0).reshape(CA, BC).sum(axis=0)
        out[c * BC : (c + 1) * BC] = s + bias_term - pad_term
    return out


# revision 7
# speedup vs baseline: 2.7923x; 2.4589x over previous
"""Behler-Parrinello NN (moe_routing) Trainium2 kernel.

Strategy:
  - Data-parallel over batch B=512 across 8 NeuronCores (64 rows each).
  - Atoms are routed: sorted by type into type-pure "chunks" of 8 atoms
    (x 64 batch rows = 512 tokens), padded with zero-atoms to chunk
    boundaries.  Each chunk runs through its own type's MLP weights, so
    compute is 1/T of the reference's all-types evaluation.
  - Per 128x512 SBUF tile we stack two chunks (top/bottom 64 partitions)
    and use tensor-engine tile_position packing so two 64x64 matmuls run
    concurrently in the 128x128 array.
  - Layer 3 (H2 -> 1) matmuls accumulate into a single persistent PSUM
    bank across the whole kernel, which implements the sum over atoms for
    free; the per-batch reduction finishes on the host (tiny).
  - silu(W x + b) is fused on ScalarE via activation(Silu, bias=...).
"""

import os
import sys

for _p in ("/opt/trn_rl_repo", "/root/.axon_site/_ro/trn_rl_repo"):
    if os.path.isdir(_p) and _p not in sys.path:
        sys.path.insert(0, _p)

import numpy as np

import concourse.bass as bass
import concourse.tile as tile
from concourse import bacc, mybir
from concourse.bass import ts
from concourse.bass_utils import run_bass_kernel_spmd

B, N, F, T, H1, H2 = 512, 2048, 64, 4, 64, 32
NCORES = 8
BC = B // NCORES          # 64 batch rows per core
CA = 8                    # atoms per chunk; CA * BC = 512 tokens per chunk
F32 = mybir.dt.float32
BF16 = mybir.dt.bfloat16

# test.py can read these after a traced run
LAST_EXEC_NS = None
LAST_RESULTS = None


def _ensure_ntff_hook():
    """This image's antenv lacks axon_hooks; synthesize it and install the
    ctypes NTFF profile hook from trn_agent_boot so trace=True works."""
    import importlib.util
    import types

    if importlib.util.find_spec("antenv.axon_hooks") is not None:
        return
    import antenv

    mod = types.ModuleType("antenv.axon_hooks")
    mod._hook = None
    mod.set_axon_ntff_profile_hook = lambda h: setattr(mod, "_hook", h)
    mod.get_axon_ntff_profile_hook = lambda: mod._hook
    sys.modules["antenv.axon_hooks"] = mod
    antenv.axon_hooks = mod
    try:
        from trn_agent_boot.trn_boot import _ntff_profile_via_ctypes

        mod._hook = _ntff_profile_via_ctypes("/opt/axon/libaxon_pjrt.so")
    except Exception as e:  # degrade to no-trace
        print(f"ntff hook install failed: {e}", file=sys.stderr)


def _chunk_schedule(an):
    """Sort atoms by type, pad each type to a chunk multiple, pad chunk count
    to a multiple of 4 (one quad = 4 chunks).  Returns (slots, ctype,
    counts, pad_counts): slots is [nchunks*CA] atom indices with -1 = pad."""
    order = np.argsort(an, kind="stable")
    counts = np.bincount(an, minlength=T).astype(np.int64)
    slots = []
    ctype = []
    pad_counts = np.zeros(T, dtype=np.int64)
    pos = 0
    for t in range(T):
        idx = order[pos : pos + counts[t]]
        pos += counts[t]
        nch = (counts[t] + CA - 1) // CA
        padded = np.full(nch * CA, -1, dtype=np.int64)
        padded[: counts[t]] = idx
        pad_counts[t] += nch * CA - counts[t]
        slots.append(padded)
        ctype.extend([t] * int(nch))
    while len(ctype) % 4 != 0:
        slots.append(np.full(CA, -1, dtype=np.int64))
        pad_counts[T - 1] += CA
        ctype.append(T - 1)
    return np.concatenate(slots), np.array(ctype, dtype=np.int64), counts, pad_counts


def gen_bass(nchunks, ctype):
    """Build the per-core Bass kernel.  ctype (len nchunks, multiple of 4)
    is baked in at compile time."""
    npairs = nchunks // 2
    nquads = nchunks // 4
    Silu = mybir.ActivationFunctionType.Silu

    nc = bacc.Bacc(None, target_bir_lowering=False)
    xt3 = nc.dram_tensor("xt3", [npairs, 128, CA * BC], BF16, kind="ExternalInput")
    w0d = nc.dram_tensor("w0s", [128, 16 * 128], BF16, kind="ExternalInput")
    w1d = nc.dram_tensor("w1s", [128, 16 * 64], BF16, kind="ExternalInput")
    w2d = nc.dram_tensor("w2s", [128, 16 * 64], BF16, kind="ExternalInput")
    b0d = nc.dram_tensor("b0p", [128, npairs], F32, kind="ExternalInput")
    b1d = nc.dram_tensor("b1q", [128, nquads], F32, kind="ExternalInput")
    outd = nc.dram_tensor("out", [4, CA * BC], F32, kind="ExternalOutput")

    with tile.TileContext(nc) as tc:
        with (
            tc.tile_pool(name="consts", bufs=1) as cpool,
            tc.tile_pool(name="xp", bufs=6) as xpool,
            tc.tile_pool(name="h1p", bufs=3) as h1pool,
            tc.tile_pool(name="h2p", bufs=3) as h2pool,
            tc.tile_pool(name="outp", bufs=1) as opool,
            tc.tile_pool(name="ps1", bufs=2, space="PSUM") as ps1pool,
            tc.tile_pool(name="ps2", bufs=2, space="PSUM") as ps2pool,
            tc.tile_pool(name="ps3", bufs=2, space="PSUM") as ps3pool,
        ):
            w0t = cpool.tile([128, 16 * 128], BF16)
            nc.sync.dma_start(w0t[:], w0d[:])
            w1t = cpool.tile([128, 16 * 64], BF16)
            nc.sync.dma_start(w1t[:], w1d[:])
            w2t = cpool.tile([128, 16 * 64], BF16)
            nc.sync.dma_start(w2t[:], w2d[:])
            b0t = cpool.tile([128, npairs], F32)
            nc.sync.dma_start(b0t[:], b0d[:])
            b1t = cpool.tile([128, nquads], F32)
            nc.sync.dma_start(b1t[:], b1d[:])

            # SBUF accumulator for the atom-sum; row blocks of 32, only the
            # first row of each block is nonzero (w2 lanes are zero-padded)
            acc = opool.tile([128, 512], F32)
            nc.vector.memset(acc[:], 0.0)

            for q in range(nquads):
                tA, tB, tC, tD = (int(t) for t in ctype[4 * q : 4 * q + 4])
                x0 = xpool.tile([128, 512], BF16, tag="x")
                nc.sync.dma_start(x0[:], xt3[2 * q])
                x1 = xpool.tile([128, 512], BF16, tag="x")
                nc.sync.dma_start(x1[:], xt3[2 * q + 1])

                # combo ids: block-diag weights [[W_top, 0], [0, W_bot]]
                cAB = tA * 4 + tB
                cCD = tC * 4 + tD

                # L1: one K=128, M=128 matmul per chunk-pair
                ps1 = ps1pool.tile([128, 1024], F32)
                nc.tensor.matmul(ps1[:, 0:512], w0t[:, ts(cAB, 128)],
                                 x0[:, :], start=True, stop=True,
                                 tile_position=(0, 0))
                nc.tensor.matmul(ps1[:, 512:1024], w0t[:, ts(cCD, 128)],
                                 x1[:, :], start=True, stop=True,
                                 tile_position=(0, 0))

                h1 = h1pool.tile([128, 1024], BF16)
                if (tA, tB) == (tC, tD):
                    nc.scalar.activation(h1[:], ps1[:], Silu,
                                         bias=b0t[:, 2 * q : 2 * q + 1])
                else:
                    nc.scalar.activation(h1[:, 0:512], ps1[:, 0:512], Silu,
                                         bias=b0t[:, 2 * q : 2 * q + 1])
                    nc.scalar.activation(h1[:, 512:1024], ps1[:, 512:1024], Silu,
                                         bias=b0t[:, 2 * q + 1 : 2 * q + 2])

                # L2: one K=128, M=64 matmul per pair; quarters c0,c1,c2,c3
                ps2 = ps2pool.tile([128, 512], F32)
                nc.tensor.matmul(ps2[0:64, :], w1t[:, ts(cAB, 64)],
                                 h1[:, 0:512], start=True, stop=True,
                                 tile_position=(0, 0))
                nc.tensor.matmul(ps2[64:128, :], w1t[:, ts(cCD, 64)],
                                 h1[:, 512:1024], start=True, stop=True,
                                 tile_position=(0, 64))

                h2 = h2pool.tile([128, 512], BF16)
                nc.scalar.activation(h2[:], ps2[:], Silu, bias=b1t[:, q : q + 1])

                # L3: one K=64, M=64 matmul per pair (w2 zero-padded blocks);
                # nonzero output rows: 0, 32, 64, 96
                ps3 = ps3pool.tile([128, 512], F32, tag="ps3")
                nc.tensor.matmul(ps3[0:64, :], w2t[0:64, ts(cAB, 64)],
                                 h2[0:64, :], start=True, stop=True,
                                 tile_position=(0, 0))
                nc.tensor.matmul(ps3[64:128, :], w2t[64:128, ts(cCD, 64)],
                                 h2[64:128, :], start=True, stop=True,
                                 tile_position=(64, 64))
                nc.vector.tensor_add(out=acc[:], in0=acc[:], in1=ps3[:])

            for i, p in enumerate((0, 32, 64, 96)):
                nc.sync.dma_start(outd[i : i + 1, :], acc[p : p + 1, :])
    nc.finalize()
    return nc


def _prep_core_x(x_c, slots, mask, npairs):
    """[BC, N, F] full-precision batch shard -> [npairs, 128, CA*BC] tiles.
    Tile p partition h*F+f, column a*BC+b = x_c[b, slots[(2p+h)*CA+a], f]."""
    xg = x_c[:, np.where(mask, slots, 0), :]          # [BC, NS, F]
    xg[:, ~mask, :] = 0.0
    nchunks = slots.shape[0] // CA
    xg = np.ascontiguousarray(xg.transpose(1, 2, 0))  # [NS, F, BC]
    xg = xg.reshape(nchunks, CA, F, BC).transpose(0, 2, 1, 3)  # [ch, F, CA, BC]
    return np.ascontiguousarray(xg).reshape(npairs, 2 * F, CA * BC)


def kernel(x, atomic_numbers, w0, b0, w1, b1, w2, b2, trace=False):
    global LAST_EXEC_NS, LAST_RESULTS
    x = np.asarray(x, dtype=np.float32)
    an = np.asarray(atomic_numbers).astype(np.int64)
    w0 = np.asarray(w0, dtype=np.float32)
    b0 = np.asarray(b0, dtype=np.float32)
    w1 = np.asarray(w1, dtype=np.float32)
    b1 = np.asarray(b1, dtype=np.float32)
    w2 = np.asarray(w2, dtype=np.float32)
    b2 = np.asarray(b2, dtype=np.float32)

    slots, ctype, counts, pad_counts = _chunk_schedule(an)
    nchunks = len(ctype)
    npairs, nquads = nchunks // 2, nchunks // 4
    mask = slots >= 0

    # --- device-side weight/bias layouts (shared across cores) ---
    import ml_dtypes

    bf16 = ml_dtypes.bfloat16
    w0s = np.zeros((128, 16 * 128), dtype=np.float32)
    w1s = np.zeros((128, 16 * 64), dtype=np.float32)
    w2s = np.zeros((128, 16 * 64), dtype=np.float32)
    for tt in range(T):
        for tb in range(T):
            c = tt * 4 + tb
            w0s[0:64, c * 128 : c * 128 + 64] = w0[tt].T
            w0s[64:128, c * 128 + 64 : c * 128 + 128] = w0[tb].T
            w1s[0:64, c * 64 : c * 64 + 32] = w1[tt].T
            w1s[64:128, c * 64 + 32 : c * 64 + 64] = w1[tb].T
            for half in (0, 64):
                w2s[half : half + 32, c * 64] = w2[tt, 0, :]
                w2s[half + 32 : half + 64, c * 64 + 32] = w2[tb, 0, :]
    b0p = np.zeros((128, npairs), dtype=np.float32)
    for p in range(npairs):
        b0p[0:64, p] = b0[ctype[2 * p]]
        b0p[64:128, p] = b0[ctype[2 * p + 1]]
    b1q = np.zeros((128, nquads), dtype=np.float32)
    for q in range(nquads):
        tA, tB, tC, tD = ctype[4 * q : 4 * q + 4]
        b1q[0:32, q] = b1[tA]
        b1q[32:64, q] = b1[tB]
        b1q[64:96, q] = b1[tC]
        b1q[96:128, q] = b1[tD]

    shared = {"w0s": w0s.astype(bf16), "w1s": w1s.astype(bf16),
              "w2s": w2s.astype(bf16), "b0p": b0p, "b1q": b1q}
    in_maps = []
    for c in range(NCORES):
        xt3 = _prep_core_x(x[c * BC : (c + 1) * BC], slots, mask, npairs).astype(bf16)
        in_maps.append({"xt3": xt3, **shared})

    if trace:
        _ensure_ntff_hook()
    nc = gen_bass(nchunks, ctype)
    res = run_bass_kernel_spmd(nc, in_maps, core_ids=list(range(NCORES)),
                               trace=trace)
    LAST_EXEC_NS = res.exec_time_ns
    LAST_RESULTS = res

    # --- host-side unshard + tiny corrections ---
    # device out = sum over streamed tokens of w2 . h2(token); pads
    # contribute e0[t] = w2[t] . silu(w1[t] silu(b0[t]) + b1[t]); real atoms
    # still owe their +b2[t].
    def _silu(v):
        return v / (1.0 + np.exp(-v))

    e0 = np.zeros(T, dtype=np.float64)
    for t in range(T):
        h1v = _silu(b0[t].astype(np.float64))
        h2v = _silu(w1[t].astype(np.float64) @ h1v + b1[t])
        e0[t] = w2[t, 0] @ h2v
    bias_term = float((counts * b2[:, 0].astype(np.float64)).sum())
    pad_term = float((pad_counts * e0).sum())

    out = np.empty(B, dtype=np.float32)
    for c in range(NCORES):
        dev = res.results[c]["out"]                   # [4, CA*BC]
        s = dev.sum(axis=0).reshape(CA, BC).sum(axis=0)
        out[c * BC : (c + 1) * BC] = s + bias_term - pad_term
    return out
